# revision 3
# baseline (speedup 1.0000x reference)
"""LocalGOCor (PWC-Net local correlation, radius 4) on 8 Trainium2 NeuronCores.

scores[b, d, y, x] = sum_c (gain * f[b,c,y,x]) * q_zeropad[b, c, y+dy, x+dx]
for d = dy*9+dx, dy/dx in [0,9)  (displacement dy-4, dx-4).

Strategy (data-parallel over batch, 2 samples per core):
  - Image tiled into 8x8 pixel blocks (M=64).  Per block one TensorE
    matmul: lhsT = F[c, 64 pixels] (K=C=128), rhs = Q window
    [c, 16y x 16x] read straight out of the q row tile with a 2D strided
    AP.  PSUM[p=(ys,xs), (wy,wx)] holds all pairwise dots; the 81 useful
    displacement values per pixel live at (wy,wx) = (ys+dy, xs+dx).
    The 16x16 window (256/pixel vs 384 for 8x16 blocks) minimizes output
    HBM traffic, which dominates.  Two blocks share each PSUM bank via
    PE column tiling (tile_position (0,0)/(0,64)).
  - Inputs are downcast to bf16 on the host (gain folded into F); matmuls
    run in bf16.  No device-side zero padding at all: x-edge blocks use
    narrowed matmuls, y-halo rows are simply left unloaded, and the host
    zeroes the out-of-image displacement stripes after extraction (the
    garbage lands exactly there).
  - ACT/DVE copy PSUM->SBUF (downcast bf16) into one [128, 32KB] tile per
    64-row chunk; a single ~4.2 MB DMA (issued from the otherwise-idle
    GPSIMD queue so it never blocks input loads on SP) writes it out.
    The band ("diagonal") extraction is a zero-copy numpy as_strided
    shear on the host during unsharding.
"""

import numpy as np

B, C, H, W = 16, 128, 128, 128
R = 4
ND = 2 * R + 1            # 9 displacements per axis
NCORES = 8
BLOC = B // NCORES        # 2 samples per core
BY, BX = 8, 8             # pixels per block -> M = 64
WY, WX = BY + 2 * R, BX + 2 * R   # 16, 16 query window
NWIN = WY * WX            # 256
YBLK = 64                 # image rows per chunk
NYC = H // YBLK           # 2
QROWS = YBLK + 2 * R      # 72
NYSUB = YBLK // BY        # 8 y-subblocks per chunk
NXB = W // BX             # 16 x-blocks

_CACHE = {}


def _build():
    import concourse.bacc as bacc
    import concourse.tile as tile
    import concourse.mybir as mybir
    from contextlib import ExitStack

    nc = bacc.Bacc(
        "TRN2",
        target_bir_lowering=False,
        debug=False,
        enable_asserts=False,
        num_devices=NCORES,
    )
    f32 = mybir.dt.float32
    bf16 = mybir.dt.bfloat16

    # f host-packed (pre-scaled by gain): [BLOC, C, NYC, NYSUB*NXB, BY*BX]
    f_dram = nc.dram_tensor("f", [BLOC, C, NYC, NYSUB * NXB, BY * BX], bf16,
                            kind="ExternalInput").ap()
    q_dram = nc.dram_tensor("q", [BLOC, C, H, W], bf16, kind="ExternalInput").ap()
    o_dram = nc.dram_tensor(
        "out", [BLOC, NYC, C, NYSUB, NXB // 2, NWIN], bf16,
        kind="ExternalOutput").ap()

    with tile.TileContext(nc) as tc, ExitStack() as ctx:
        qpool = ctx.enter_context(tc.tile_pool(name="qpool", bufs=2))
        fpool = ctx.enter_context(tc.tile_pool(name="fpool", bufs=2))
        opool = ctx.enter_context(tc.tile_pool(name="opool", bufs=2))
        pspool = ctx.enter_context(tc.tile_pool(name="pspool", bufs=2, space="PSUM"))

        for b in range(BLOC):
            for yc in range(NYC):
                ql = qpool.tile([C, QROWS, W], bf16, tag="ql")
                ft = fpool.tile([C, NYSUB * NXB, BY * BX], bf16, tag="ft")
                # per x-pair layout: [part, y0i, xpair(k,h), win]
                ot = opool.tile([C, NYSUB, NXB // 2, NWIN], bf16, tag="ot")

                nc.sync.dma_start(out=ft[:, :, :], in_=f_dram[b, :, yc, :, :])

                # ql row r <-> real row yc*YBLK + r - 4; out-of-image halo
                # rows are left stale (host zeroes those output stripes)
                r_lo = yc * YBLK - R
                r_hi = yc * YBLK + YBLK + R
                lo_clip, hi_clip = max(r_lo, 0), min(r_hi, H)
                t_lo = lo_clip - r_lo
                nc.sync.dma_start(
                    out=ql[:, t_lo:t_lo + (hi_clip - lo_clip), :],
                    in_=q_dram[b, :, lo_clip:hi_clip, :],
                )

                for y0i in range(NYSUB):
                    y0 = y0i * BY
                    # 16 x-blocks -> one full-PSUM tile: 4 banks x 2
                    # half-banks x 2 partition-halves
                    pt = pspool.tile([C, 4, 2, WY, WX], f32, tag="pt")
                    for j in range(NXB):
                        k, h, ph = j // 4, (j % 4) // 2, j % 2
                        blk = y0i * NXB + j
                        c_lo = BX * j - R
                        cl, ch = max(c_lo, 0), min(c_lo + WX, W)
                        p_lo = cl - c_lo
                        nc.tensor.matmul(
                            pt[64 * ph:64 * ph + 64, k, h, :, p_lo:p_lo + (ch - cl)],
                            ft[:, blk, :],
                            ql[:, y0:y0 + WY, cl:ch],
                            start=True, stop=True,
                            tile_position=(0, 64 * ph),
                        )
                    src = pt[:, :, :, :, :]
                    if y0i % 2 == 0:
                        nc.scalar.copy(ot[:, y0i, :, :], src)
                    else:
                        nc.vector.tensor_copy(ot[:, y0i, :, :], src)

                nc.gpsimd.dma_start(out=o_dram[b, yc, :, :, :, :],
                                    in_=ot[:, :, :, :])

    nc.compile()
    return nc


def _get_nc():
    if "nc" not in _CACHE:
        _CACHE["nc"] = _build()
    return _CACHE["nc"]


def pack_f(f: np.ndarray, gain: float) -> np.ndarray:
    """[Bany, C, H, W] f32 -> gain-scaled bf16
    [Bany, C, NYC, NYSUB*NXB, BY*BX] block-contiguous."""
    import ml_dtypes
    n = f.shape[0]
    v = (f * np.float32(gain)).astype(ml_dtypes.bfloat16)
    v = v.reshape(n, C, NYC, NYSUB, BY, NXB, BX)
    v = v.transpose(0, 1, 2, 3, 5, 4, 6)   # b,c,yc,y0i,j,ys,xs
    return np.ascontiguousarray(v.reshape(n, C, NYC, NYSUB * NXB, BY * BX))


def _extract(O: np.ndarray) -> np.ndarray:
    """O: [B, NYC, C(part), NYSUB, NXB//2, NWIN] bf16 -> [B, 81, H, W] f32."""
    Of = np.ascontiguousarray(O.astype(np.float32))
    # part = (ph, ys, xs); xpair = (k, h); win = (wy, wx)
    V = Of.reshape(B, NYC, 2, BY, BX, NYSUB, 4, 2, WY, WX)
    sb, syc, sph, sys, sxs, sy0, sk, sh, swy, swx = V.strides
    T = np.lib.stride_tricks.as_strided(
        V,
        shape=(B, ND, ND, NYC, NYSUB, BY, 4, 2, 2, BX),
        strides=(sb, swy, swx, syc, sy0, sys + swy, sk, sh, sph, sxs + swx),
    )
    out = np.ascontiguousarray(T.reshape(B, ND * ND, H, W))
    # zero the out-of-image displacement stripes (device wrote garbage
    # there: stale SBUF halo rows / unwritten PSUM edge columns)
    for dy in range(ND):
        for dx in range(ND):
            d = dy * ND + dx
            if dy < R:
                out[:, d, 0:R - dy, :] = 0.0
            elif dy > R:
                out[:, d, H - (dy - R):H, :] = 0.0
            if dx < R:
                out[:, d, :, 0:R - dx] = 0.0
            elif dx > R:
                out[:, d, :, W - (dx - R):W] = 0.0
    return out


def make_in_maps(f: np.ndarray, q: np.ndarray, gain: float):
    import ml_dtypes
    fp = pack_f(f, gain)
    qb = q.astype(ml_dtypes.bfloat16)
    return [
        {"f": fp[BLOC * c:BLOC * (c + 1)], "q": qb[BLOC * c:BLOC * (c + 1)]}
        for c in range(NCORES)
    ]


def kernel(**inputs) -> np.ndarray:
    from concourse.bass_utils import run_bass_kernel_spmd

    f = np.ascontiguousarray(np.asarray(inputs["reference_feat"], dtype=np.float32))
    q = np.ascontiguousarray(np.asarray(inputs["query_feat"], dtype=np.float32))
    gain = float(np.asarray(inputs["init_gain"]).reshape(-1)[0])

    nc = _get_nc()
    in_maps = make_in_maps(f, q, gain)
    res = run_bass_kernel_spmd(nc, in_maps, core_ids=list(range(NCORES)))

    O = np.stack([res.results[c]["out"] for c in range(NCORES)])
    O = O.reshape(B, NYC, C, NYSUB, NXB // 2, NWIN)
    return _extract(O)
